# revision 2
# baseline (speedup 1.0000x reference)
"""Trainium2 Bass kernel for hierarchical LSTM (nn_ABHUE_66185446032058).

Single-core design (v2).  The previous 8-core SPMD kernel spent ~1.5 s/call on
host work: ~40 MB of per-core inputs concatenated + shipped through the axon
tunnel (~25-55 MB/s) to 8 devices, a fresh jax.jit every call, and an
AllGather.  Device compute is < 1 ms, so everything moves to ONE core:

 - x ships as float16 (3.2 MB), transposed on-device (PE transposes).
 - Weights are preprocessed once, cached on-device keyed by a content hash.
 - The jax.jit callable is built once per process and reused.
 - No collectives.

Device program:
  Phase A: per-sentence gates_ih = x_s @ WihT (+bias via ones row), written
    f32 PSUM -> DRAM gihD[word, chain, 800].
  Phase B: 63 chains (chain 31 = the main-LSTM chain) advance 128 steps in
    lockstep.  Chain 31 rides the ctx matmul and adds (Whh_main - Whh_ctx)
    through dedicated K-rows, so ctx hT copies stay full-width.
    Gate columns are permuted (i,g | f,o) so the (i,g) tanh can start after
    the first PSUM bank's matmuls.
  Phase C: prev/post sentence-level LSTMs as a fused 2-chain recurrence of
    the same shape (32 steps), then the final FC.

Numerics: single-tanh trick (sigmoid(x) = 0.5 + 0.5 tanh(x/2)) with 0.5
scales folded into weights; device state h' = 2h, X = 2c.  Matmuls are f32r
(exact f32 bits) except phase A which is f16 (x and Wih); gih is f32.
"""

import sys
import hashlib
import numpy as np

if "/opt/trn_rl_repo" not in sys.path:
    sys.path.insert(0, "/opt/trn_rl_repo")

S, W, E, H = 63, 128, 200, 200
G4 = 4 * H
MID = (S - 1) // 2          # 31
NCH = 63                    # chains: 0..62, chain MID is the main LSTM
PREV_LEN = MID + 1          # 32
RING = 4

_CACHE = {}

# gate column permutation: original (i f g o) -> (i g | f o)
_COLPERM = np.concatenate([
    np.arange(0, 200), np.arange(400, 600),      # bank1: i, g
    np.arange(200, 400), np.arange(600, 800),    # bank2: f, o
])


def _build(dbg=False):
    import concourse.bass as bass
    import concourse.bacc as bacc
    import concourse.tile as tile
    import concourse.mybir as mybir

    f32 = mybir.dt.float32
    f32r = mybir.dt.float32r
    f16 = mybir.dt.float16
    AF = mybir.ActivationFunctionType
    OP = mybir.AluOpType

    nc = bacc.Bacc(None, target_bir_lowering=False, num_devices=1)

    def din(name, shape, dt=None):
        return nc.dram_tensor(name, list(shape), dt or f32r, kind="ExternalInput")

    # ---- per-call inputs ----
    x16 = din("x16", [S, W, E], f16)
    h0a = din("h0a", [128, NCH])      # 2*h0s.T rows 0:128 (all chains)
    h0b = din("h0b", [72, NCH])       # rows 128:200
    h0m1 = din("h0m1", [128, 1])      # main chain h0 rows 0:128
    h0m2 = din("h0m2", [72, 1])       # rows 128:200
    x0 = din("x0", [NCH, H], f32)          # X = 2*c0s
    hcp1 = din("hcp1", [128, 1])      # 2*h_prev0 rows 0:128
    hcp2 = din("hcp2", [72, 1])
    hcq1 = din("hcq1", [128, 1])      # 2*h_post0 rows 0:128
    hcq2 = din("hcq2", [72, 1])       # rows 128:200
    xc0 = din("xc0", [2, H], f32)          # [2*c_prev0; 2*c_post0]

    # ---- cached (weight) inputs ----
    wihC1 = din("wihC1", [128, G4], f16)
    wihC2 = din("wihC2", [73, G4], f16)    # row 72 = bias
    wihM1 = din("wihM1", [128, G4], f16)
    wihM2 = din("wihM2", [73, G4], f16)
    whhC1 = din("whhC1", [128, G4])        # ctx Whh rows 0:128
    whhC2 = din("whhC2", [72, G4])         # rows 128:200
    whhD1 = din("whhD1", [128, G4])        # (main - ctx) rows 0:128
    whhD2 = din("whhD2", [72, G4])         # rows 128:200
    whhP1 = din("whhP1", [128, G4])        # prev Whh rows 0:128
    whhP2 = din("whhP2", [72, G4])
    whhQ1 = din("whhQ1", [128, G4])        # post Whh rows 0:128
    whhQ2 = din("whhQ2", [72, G4])         # rows 128:200
    wihP1 = din("wihP1", [128, G4])
    wihP2 = din("wihP2", [73, G4])         # row 72 = bias
    wihQ1 = din("wihQ1", [128, G4])
    wihQ2 = din("wihQ2", [73, G4])
    fc0 = din("fc0", [128, 200])
    fc1 = din("fc1", [72, 200])
    fc2 = din("fc2", [128, 200])
    fc3 = din("fc3", [72, 200])
    fc4 = din("fc4", [1, 200])             # bfc
    id63 = din("id63", [S, S], f32)
    id63r = din("id63r", [S, S])
    id128 = din("id128", [128, 128], f16)
    onesW = din("onesW", [1, 128], f16)
    ones1 = din("ones1", [1, 1])
    ones63 = din("ones63", [1, NCH])

    outD = nc.dram_tensor("out", [1, 200], f32, kind="ExternalOutput")
    if dbg:
        dbgGih = nc.dram_tensor("dbgGih", [W, NCH, G4], f32, kind="ExternalOutput")
        dbgSE = nc.dram_tensor("dbgSE", [NCH, H], f32, kind="ExternalOutput")
        dbgGP = nc.dram_tensor("dbgGP", [PREV_LEN, 2, G4], f32, kind="ExternalOutput")
        dbgHC = nc.dram_tensor("dbgHC", [128, 4], f32, kind="ExternalOutput")

    with tile.TileContext(nc) as tc:
        with tc.tile_pool(name="dram", bufs=1, space="DRAM") as dpool, \
             tc.tile_pool(name="const", bufs=1) as cpool, \
             tc.tile_pool(name="work", bufs=2) as wpool, \
             tc.tile_pool(name="psum", bufs=2, space="PSUM") as ppool, \
             tc.tile_pool(name="psumt", bufs=2, space="PSUM") as tpool:

            gihD = dpool.tile([W, NCH, G4], f32r, tag="gihD", name="gihD")
            gihCD = dpool.tile([PREV_LEN, 2, G4], f32r, tag="gihCD", name="gihCD")

            def load(t, tag, dt=None):
                tl = cpool.tile(list(t.shape), dt or t.dtype, tag=tag)
                nc.sync.dma_start(tl[:], t[:])
                return tl

            # static SBUF loads
            xAll = cpool.tile([128, S, E], f16, tag="xAll")
            nc.sync.dma_start(xAll[:], x16[:].rearrange("s w e -> w s e"))
            wC1_s = load(wihC1, "wC1"); wC2_s = load(wihC2, "wC2")
            wM1_s = load(wihM1, "wM1"); wM2_s = load(wihM2, "wM2")
            id128_s = load(id128, "id128")
            id63_s = load(id63, "id63")
            T1r = load(whhC1, "T1r")
            C1r = load(whhP1, "C1r")
            wP1_s = load(wihP1, "wP1"); wP2_s = load(wihP2, "wP2")
            wQ1_s = load(wihQ1, "wQ1"); wQ2_s = load(wihQ2, "wQ2")
            fc_s = [load(t, f"fc{i}") for i, t in enumerate((fc0, fc1, fc2, fc3, fc4))]
            ones1_s = load(ones1, "ones1")

            # ---------------- Phase A: gates_ih for all words ----------------
            for c in range(NCH):
                TXa = tpool.tile([128, 128], f16, tag="TPa", name="TPa")
                nc.tensor.transpose(TXa[:], xAll[:, c, 0:128], id128_s[:])
                TXb = tpool.tile([72, 128], f16, tag="TPb", name="TPb")
                nc.tensor.transpose(TXb[:], xAll[:, c, 128:200], id128_s[:])
                xT1 = wpool.tile([128, 128], f16, tag="xT1", name="xT1")
                nc.vector.tensor_copy(xT1[:], TXa[:])
                xT2 = wpool.tile([73, 128], f16, tag="xT2", name="xT2")
                nc.scalar.copy(xT2[0:72, :], TXb[:])
                nc.sync.dma_start(xT2[72:73, :], onesW[:])
                W1, W2 = (wM1_s, wM2_s) if c == MID else (wC1_s, wC2_s)
                ps = ppool.tile([128, 1024], f32, tag="psA", name="psA", bufs=1)
                for nb in range(2):
                    po = slice(nb * 512, nb * 512 + 400)
                    sl = slice(nb * 400, nb * 400 + 400)
                    nc.tensor.matmul(ps[:, po], xT1[:], W1[:, sl],
                                     start=True, stop=False)
                    nc.tensor.matmul(ps[:, po], xT2[:], W2[:, sl],
                                     start=False, stop=True)
                gA = wpool.tile([128, G4], f32r, tag="gA", name="gA")
                nc.vector.tensor_copy(gA[:, 0:400], ps[:, 0:400])
                nc.scalar.copy(gA[:, 400:800], ps[:, 512:912])
                nc.sync.dma_start(gihD[:, c, :], gA[:])

            # ---------------- generic fused recurrence ----------------
            # tiles_h: list of (lhsT tile pair, rhs tile or ring list)
            def step(t, M, hT_tiles, rhs_of, Gp_tag, X_tiles, ident, nsub,
                     h_writes, ring_dma):
                """One LSTM step for M chains.
                hT_tiles: dict idx->pair of lhsT tiles (ping/pong)
                rhs_of(i, t): rhs tile for K-tile i
                h_writes(TPa, TPb, cur): copy transposed h into hT tiles [cur]
                ring_dma(t): issue gih DMAs for step t
                """
                if ring_dma is not None:
                    ring_dma(t)
                cur, nxt = t % 2, (t + 1) % 2
                Gp = ppool.tile([M, 1024], f32, tag="psA", name="psA",
                                bufs=1)
                # bank1 = (i, g) first, then bank2 = (f, o)
                for nb in range(2):
                    po = slice(nb * 512, nb * 512 + 400)
                    sl = slice(nb * 400, nb * 400 + 400)
                    for i in range(nsub):
                        nc.tensor.matmul(
                            Gp[:, po], hT_tiles[i][cur][:], rhs_of(i, t)[:, sl],
                            start=(i == 0), stop=(i == nsub - 1))
                Tig = wpool.tile([M, 400], f32, tag=f"Tig{Gp_tag}", name=f"Tig{Gp_tag}")
                nc.scalar.activation(Tig[:], Gp[:, 0:400], AF.Tanh)
                Tf = wpool.tile([M, 200], f32, tag=f"Tf{Gp_tag}", name=f"Tf{Gp_tag}")
                nc.scalar.activation(Tf[:], Gp[:, 512:712], AF.Tanh)
                To = wpool.tile([M, 200], f32, tag=f"To{Gp_tag}", name=f"To{Gp_tag}")
                nc.scalar.activation(To[:], Gp[:, 712:912], AF.Tanh)
                u = wpool.tile([M, 200], f32, tag=f"u{Gp_tag}", name=f"u{Gp_tag}")
                nc.vector.scalar_tensor_tensor(u[:], Tig[:, 0:200], 1.0,
                                               Tig[:, 200:400], OP.add, OP.mult)
                a = wpool.tile([M, 200], f32, tag=f"a{Gp_tag}", name=f"a{Gp_tag}")
                nc.vector.scalar_tensor_tensor(a[:], Tf[:], 1.0,
                                               X_tiles[cur][:], OP.add, OP.mult)
                Xn = X_tiles[nxt]
                nc.vector.scalar_tensor_tensor(Xn[:], a[:], 0.5, u[:],
                                               OP.mult, OP.add)
                wt = wpool.tile([M, 200], f32, tag=f"wt{Gp_tag}", name=f"wt{Gp_tag}")
                nc.scalar.activation(wt[:], Xn[:], AF.Tanh, scale=0.5)
                h2 = wpool.tile([M, 200], f32, tag=f"h2{Gp_tag}", name=f"h2{Gp_tag}")
                nc.vector.scalar_tensor_tensor(h2[:], To[:], 1.0, wt[:],
                                               OP.add, OP.mult)
                TPa = tpool.tile([128, M], f32, tag="TPa", name="TPa")
                nc.tensor.transpose(TPa[:], h2[:, 0:128], ident)
                TPb = tpool.tile([72, M], f32, tag="TPb", name="TPb")
                nc.tensor.transpose(TPb[:], h2[:, 128:200], ident)
                h_writes(TPa, TPb, nxt)
                return h2, TPa, TPb

            # ---------------- Phase B ----------------
            # lhsT K-tiles (double buffered):
            # T1h [128, 63]: ctx hT rows 0:128 (all chains)
            # T2h [128, 63]: rows 0:72 ctx hT 128:200; rows 72:128 = id[0:56]
            # T3h [128, 63]: main hT rows 0:128 (col 31 only)
            # T4h [79, 63]: rows 0:72 main hT 128:200 (col 31); rows 72:79 = id[56:63]
            T1h = [cpool.tile([128, NCH], f32r, tag=f"T1h{i}", name=f"T1h{i}") for i in range(2)]
            T2h = [cpool.tile([128, NCH], f32r, tag=f"T2h{i}", name=f"T2h{i}") for i in range(2)]
            T3h = [cpool.tile([128, NCH], f32r, tag=f"T3h{i}", name=f"T3h{i}") for i in range(2)]
            T4h = [cpool.tile([79, NCH], f32r, tag=f"T4h{i}", name=f"T4h{i}") for i in range(2)]
            for i in range(2):
                nc.vector.memset(T3h[i][:].bitcast(f32), 0.0)
                nc.vector.memset(T4h[i][:].bitcast(f32), 0.0)
                nc.sync.dma_start(T2h[i][72:128, :], id63r[0:56, :])
                nc.sync.dma_start(T4h[i][72:79, :], id63r[56:63, :])
            nc.sync.dma_start(T1h[0][:], h0a[:])
            nc.sync.dma_start(T2h[0][0:72, :], h0b[:])
            nc.sync.dma_start(T3h[0][:, MID:MID + 1], h0m1[:])
            nc.sync.dma_start(T4h[0][0:72, MID:MID + 1], h0m2[:])

            # rhs ring tiles (gih rows rewritten per step)
            T3r = load(whhD1, "T3r")
            T2r = [cpool.tile([128, G4], f32r, tag=f"T2r{r}", name=f"T2r{r}") for r in range(RING)]
            T4rr = [cpool.tile([79, G4], f32r, tag=f"T4r{r}", name=f"T4r{r}") for r in range(RING)]
            for r in range(RING):
                nc.sync.dma_start(T2r[r][0:72, :], whhC2[:])
                nc.sync.dma_start(T4rr[r][0:72, :], whhD2[:])

            def b_ring(t):
                nc.sync.dma_start(T2r[t % RING][72:128, :], gihD[t, 0:56, :])
                nc.sync.dma_start(T4rr[t % RING][72:79, :], gihD[t, 56:63, :])

            def b_rhs(i, t):
                return (T1r, T2r[t % RING], T3r, T4rr[t % RING])[i]

            def b_hw(TPa, TPb, nxt):
                nc.vector.tensor_copy(T1h[nxt][:], TPa[:])
                nc.vector.tensor_copy(T2h[nxt][0:72, :], TPb[:])
                nc.scalar.copy(T3h[nxt][:, MID:MID + 1], TPa[:, MID:MID + 1])
                nc.scalar.copy(T4h[nxt][0:72, MID:MID + 1], TPb[:, MID:MID + 1])

            XB = [cpool.tile([NCH, H], f32, tag=f"XB{i}", name=f"XB{i}") for i in range(2)]
            nc.sync.dma_start(XB[0][:], x0[:])

            bT = {0: T1h, 1: T2h, 2: T3h, 3: T4h}
            for t in range(W):
                if t < W - 1:
                    h2B, TPaF, TPbF = step(t, NCH, bT, b_rhs, "GB", XB,
                                           id63_s[:], 4, b_hw, b_ring)
                else:
                    h2B, TPaF, TPbF = step(t, NCH, bT, b_rhs, "GB", XB,
                                           id63_s[:], 4, lambda *a: None, b_ring)

            # sentence embeddings (transposed): hFa [128, 63], hFb [73, 63]
            hFa = cpool.tile([128, NCH], f32r, tag="hFa")
            nc.vector.tensor_copy(hFa[:], TPaF[:])
            hFb = cpool.tile([73, NCH], f32r, tag="hFb")
            nc.vector.tensor_copy(hFb[0:72, :], TPbF[:])
            nc.sync.dma_start(hFb[72:73, :], ones63[:])
            if dbg:
                nc.sync.dma_start(dbgGih[:], gihD[:].bitcast(f32))
                nc.sync.dma_start(dbgSE[:], h2B[:])

            # ---------------- Phase C0: sentence-level gates_ih ----------------
            for k, (lo, w1s, w2s) in enumerate(((0, wP1_s, wP2_s),
                                                (MID, wQ1_s, wQ2_s))):
                ps = ppool.tile([PREV_LEN, 1024], f32, tag="psA", name="psA", bufs=1)
                for nb in range(2):
                    po = slice(nb * 512, nb * 512 + 400)
                    sl = slice(nb * 400, nb * 400 + 400)
                    nc.tensor.matmul(ps[:, po], hFa[:, lo:lo + PREV_LEN],
                                     w1s[:, sl], start=True, stop=False)
                    nc.tensor.matmul(ps[:, po], hFb[:, lo:lo + PREV_LEN],
                                     w2s[:, sl], start=False, stop=True)
                gC = wpool.tile([PREV_LEN, G4], f32r, tag="gC", name="gC")
                nc.vector.tensor_copy(
                    gC[:].rearrange("p (b x) -> p b x", b=2),
                    ps[:].rearrange("p (b x) -> p b x", b=2)[:, :, 0:400])
                nc.sync.dma_start(gihCD[:, k, :], gC[:])

            # ---------------- Phase C: prev/post recurrence (M=2) ----------------
            # chains: col 0 = prev, col 1 = post
            # C1h [128, 2]: prev hT 0:128 (col 0)
            # C2h [74, 2]: rows 0:72 prev hT 128:200 (col 0); rows 72:74 = id2
            # C3h [128, 2]: post hT 0:128 (col 1)
            # C4h [72, 2]: post hT 128:200 (col 1)
            C1h = [cpool.tile([128, 2], f32r, tag=f"C1h{i}", name=f"C1h{i}") for i in range(2)]
            C2h = [cpool.tile([74, 2], f32r, tag=f"C2h{i}", name=f"C2h{i}") for i in range(2)]
            C3h = [cpool.tile([128, 2], f32r, tag=f"C3h{i}", name=f"C3h{i}") for i in range(2)]
            C4h = [cpool.tile([72, 2], f32r, tag=f"C4h{i}", name=f"C4h{i}") for i in range(2)]
            for i in range(2):
                nc.vector.memset(C1h[i][:].bitcast(f32), 0.0)
                nc.vector.memset(C2h[i][:].bitcast(f32), 0.0)
                nc.vector.memset(C3h[i][:].bitcast(f32), 0.0)
                nc.vector.memset(C4h[i][:].bitcast(f32), 0.0)
                nc.sync.dma_start(C2h[i][72:74, 0:2], id63r[0:2, 0:2])
            nc.sync.dma_start(C1h[0][:, 0:1], hcp1[:])
            nc.sync.dma_start(C2h[0][0:72, 0:1], hcp2[:])
            nc.sync.dma_start(C3h[0][:, 1:2], hcq1[:])
            nc.sync.dma_start(C4h[0][:, 1:2], hcq2[:])

            C3r = load(whhQ1, "C3r")
            C4r = load(whhQ2, "C4r")
            C2r = [cpool.tile([74, G4], f32r, tag=f"C2r{r}", name=f"C2r{r}") for r in range(RING)]
            for r in range(RING):
                nc.sync.dma_start(C2r[r][0:72, :], whhP2[:])

            def c_ring(t):
                nc.sync.dma_start(C2r[t % RING][72:73, :], gihCD[t:t + 1, 0, :])
                nc.sync.dma_start(C2r[t % RING][73:74, :], gihCD[MID - t:MID - t + 1, 1, :])

            def c_rhs(i, t):
                return (C1r, C2r[t % RING], C3r, C4r)[i]

            def c_hw(TPa, TPb, nxt):
                nc.vector.tensor_copy(C1h[nxt][:, 0:1], TPa[:, 0:1])
                nc.vector.tensor_copy(C2h[nxt][0:72, 0:1], TPb[:, 0:1])
                nc.scalar.copy(C3h[nxt][:, 1:2], TPa[:, 1:2])
                nc.scalar.copy(C4h[nxt][:, 1:2], TPb[:, 1:2])

            XC = [cpool.tile([2, H], f32, tag=f"XC{i}", name=f"XC{i}") for i in range(2)]
            nc.sync.dma_start(XC[0][:], xc0[:])

            cT = {0: C1h, 1: C2h, 2: C3h, 3: C4h}
            id2 = id63_s[0:2, 0:2]
            for t in range(PREV_LEN):
                if t < PREV_LEN - 1:
                    h2C, TPaC, TPbC = step(t, 2, cT, c_rhs, "GC", XC,
                                           id2, 4, c_hw, c_ring)
                else:
                    h2C, TPaC, TPbC = step(t, 2, cT, c_rhs, "GC", XC,
                                           id2, 4, lambda *a: None, c_ring)

            # ---------------- FC ----------------
            hCa = wpool.tile([128, 2], f32r, tag="hCa")
            nc.vector.tensor_copy(hCa[:], TPaC[:])
            hCb = wpool.tile([72, 2], f32r, tag="hCb")
            nc.vector.tensor_copy(hCb[:], TPbC[:])
            if dbg:
                nc.sync.dma_start(dbgGP[:], gihCD[:].bitcast(f32))
                hdbg = wpool.tile([128, 4], f32, tag="hdbg")
                nc.vector.tensor_copy(hdbg[:, 0:2], hCa[:].bitcast(f32))
                nc.vector.memset(hdbg[:, 2:4], 0.0)
                nc.vector.tensor_copy(hdbg[0:72, 2:4], hCb[:].bitcast(f32))
                nc.sync.dma_start(dbgHC[:], hdbg[:])
            psO = tpool.tile([1, 200], f32, tag="TPb", name="psO")
            chunks = [(hCa[:, 0:1], fc_s[0]), (hCb[:, 0:1], fc_s[1]),
                      (hCa[:, 1:2], fc_s[2]), (hCb[:, 1:2], fc_s[3]),
                      (ones1_s[:], fc_s[4])]
            for i, (l, r) in enumerate(chunks):
                nc.tensor.matmul(psO[:], l, r[:],
                                 start=(i == 0), stop=(i == len(chunks) - 1))
            outS = wpool.tile([1, 200], f32, tag="outS")
            nc.vector.tensor_copy(outS[:], psO[:])
            nc.sync.dma_start(outD[:], outS[:])

    nc.compile()
    return nc


# ---------------------------------------------------------------- host side

def _prep_weights(inp):
    """Preprocess weight inputs -> dict of arrays (ship-once, cacheable)."""
    sg = np.ones(G4, np.float32)
    sg[0:2 * H] = 0.5
    sg[3 * H:4 * H] = 0.5
    cp = _COLPERM

    def eff_ih(Wih, bih, bhh, in_scale):
        wT = (Wih.T * sg[None, :] * in_scale)[:, cp].astype(np.float32)
        b = ((sg * (bih + bhh))[cp]).astype(np.float32)
        return wT, b

    def eff_hh(Whh):
        return ((Whh.T * sg[None, :] * 0.5)[:, cp]).astype(np.float32)

    wtC, bC = eff_ih(inp["Wih_ctx"], inp["bih_ctx"], inp["bhh_ctx"], 1.0)
    wtM, bM = eff_ih(inp["Wih_main"], inp["bih_main"], inp["bhh_main"], 1.0)
    wtP, bP = eff_ih(inp["Wih_prev"], inp["bih_prev"], inp["bhh_prev"], 0.5)
    wtQ, bQ = eff_ih(inp["Wih_post"], inp["bih_post"], inp["bhh_post"], 0.5)
    hhC = eff_hh(inp["Whh_ctx"])
    hhD = eff_hh(inp["Whh_main"]) - hhC
    hhP = eff_hh(inp["Whh_prev"])
    hhQ = eff_hh(inp["Whh_post"])

    def split2(w, b, dt):
        return (np.ascontiguousarray(w[0:128], dtype=dt),
                np.ascontiguousarray(
                    np.concatenate([w[128:200], b[None, :]], 0), dtype=dt))

    wihC1, wihC2 = split2(wtC, bC, np.float16)
    wihM1, wihM2 = split2(wtM, bM, np.float16)
    wihP1, wihP2 = split2(wtP, bP, np.float32)
    wihQ1, wihQ2 = split2(wtQ, bQ, np.float32)

    wfcT = (inp["Wfc"].T * 0.5).astype(np.float32)          # [400, 200]
    C = np.ascontiguousarray
    return dict(
        wihC1=wihC1, wihC2=wihC2, wihM1=wihM1, wihM2=wihM2,
        whhC1=C(hhC[0:128]), whhC2=C(hhC[128:200]),
        whhD1=C(hhD[0:128]), whhD2=C(hhD[128:200]),
        whhP1=C(hhP[0:128]), whhP2=C(hhP[128:200]),
        whhQ1=C(hhQ[0:128]), whhQ2=C(hhQ[128:200]),
        wihP1=wihP1, wihP2=wihP2, wihQ1=wihQ1, wihQ2=wihQ2,
        fc0=C(wfcT[0:128]), fc1=C(wfcT[128:200]),
        fc2=C(wfcT[200:328]), fc3=C(wfcT[328:400]),
        fc4=C(inp["bfc"][None, :].astype(np.float32)),
        id63=np.eye(S, dtype=np.float32),
        id63r=np.eye(S, dtype=np.float32),
        id128=np.eye(128, dtype=np.float16),
        onesW=np.ones((1, 128), np.float16),
        ones1=np.ones((1, 1), np.float32),
        ones63=np.ones((1, NCH), np.float32),
    )


def _prep_call(inp):
    """Per-call (activation) inputs."""
    C = np.ascontiguousarray
    h2 = (2.0 * np.asarray(inp["h0s"], np.float32)).T      # [200, 63]
    hm = 2.0 * np.asarray(inp["h0s"], np.float32)[MID]
    hp = 2.0 * np.asarray(inp["h_prev0"], np.float32)
    hq = 2.0 * np.asarray(inp["h_post0"], np.float32)
    return dict(
        x16=np.asarray(inp["x"]).astype(np.float16),
        h0a=C(h2[0:128]), h0b=C(h2[128:200]),
        h0m1=C(hm[0:128, None]), h0m2=C(hm[128:200, None]),
        x0=C(2.0 * np.asarray(inp["c0s"], np.float32)),
        hcp1=C(hp[0:128, None]), hcp2=C(hp[128:200, None]),
        hcq1=C(hq[0:128, None]), hcq2=C(hq[128:200, None]),
        xc0=C(np.stack([2.0 * np.asarray(inp["c_prev0"], np.float32),
                        2.0 * np.asarray(inp["c_post0"], np.float32)])),
    )


_WEIGHT_KEYS = ("Wih_ctx", "Whh_ctx", "bih_ctx", "bhh_ctx",
                "Wih_main", "Whh_main", "bih_main", "bhh_main",
                "Wih_prev", "Whh_prev", "bih_prev", "bhh_prev",
                "Wih_post", "Whh_post", "bih_post", "bhh_post",
                "Wfc", "bfc")


def _weights_fp(inp):
    h = hashlib.blake2b(digest_size=16)
    for k in _WEIGHT_KEYS:
        a = np.ascontiguousarray(np.asarray(inp[k]))
        h.update(a.tobytes())
    return h.hexdigest()


def _get_nc(dbg=False):
    key = f"nc{dbg}"
    if key not in _CACHE:
        _CACHE[key] = _build(dbg)
    return _CACHE[key]


def _get_jit(nc, key):
    """Build (once) a cached jax.jit callable for this bass module."""
    import jax
    import concourse.mybir as mybir
    from concourse import bass2jax

    if key in _CACHE:
        return _CACHE[key]
    bass2jax.install_neuronx_cc_hook()

    in_names, out_names, out_avals = [], [], []
    partition_name = nc.partition_id_tensor.name if nc.partition_id_tensor else None
    for alloc in nc.m.functions[0].allocations:
        if not isinstance(alloc, mybir.MemoryLocationSet):
            continue
        name = alloc.memorylocations[0].name
        if alloc.kind == "ExternalInput":
            if name != partition_name:
                in_names.append(name)
        elif alloc.kind == "ExternalOutput":
            out_names.append(name)
            out_avals.append(jax.core.ShapedArray(
                tuple(alloc.tensor_shape), mybir.dt.np(alloc.dtype)))
    n_params = len(in_names)
    bind_names = list(in_names) + list(out_names)
    if partition_name is not None:
        bind_names.append(partition_name)

    def mk_body():
        # distinct function identity per executable (see note below)
        def _body(*args):
            operands = list(args)
            if partition_name is not None:
                operands.append(bass2jax.partition_id_tensor())
            outs = bass2jax._bass_exec_p.bind(
                *operands,
                out_avals=tuple(out_avals),
                in_names=tuple(bind_names),
                out_names=tuple(out_names),
                lowering_input_output_aliases=(),
                sim_require_finite=True,
                sim_require_nnan=True,
                nc=nc,
            )
            return tuple(outs)
        return _body

    donate = tuple(range(n_params, n_params + len(out_names)))
    # Running the SAME loaded executable twice in a row leaves device state
    # (semaphores/queues) unreset under the axon runtime and corrupts the
    # second run.  Running a different executable in between resets it, so
    # we keep two identical executables and alternate between calls.
    jfns = [jax.jit(mk_body(), donate_argnums=donate, keep_unused=True)
            for _ in range(2)]
    entry = dict(jfns=jfns, in_names=in_names, out_names=out_names,
                 out_avals=out_avals, n=0)
    _CACHE[key] = entry
    return entry


def _run_fast(inputs, dbg=False):
    """Cached-jit single-device execution; returns dict of outputs."""
    import jax
    nc = _get_nc(dbg)
    ent = _get_jit(nc, f"jit{dbg}")
    dev = jax.devices()[0]

    fp = _weights_fp(inputs)
    wkey = f"w{fp}"
    if wkey not in _CACHE:
        w = _prep_weights(inputs)
        _CACHE[wkey] = {k: jax.device_put(v, dev) for k, v in w.items()}
        # keep only the latest weight set on device
        for k in [k for k in _CACHE if k.startswith("w") and k != wkey]:
            del _CACHE[k]
    wdev = _CACHE[wkey]

    call = _prep_call(inputs)
    args = []
    for name in ent["in_names"]:
        if name in wdev:
            args.append(wdev[name])
        else:
            args.append(call[name])
    zeros = [np.zeros(av.shape, av.dtype) for av in ent["out_avals"]]
    jfn = ent["jfns"][ent["n"] % len(ent["jfns"])]
    ent["n"] += 1
    outs = jfn(*args, *zeros)
    return {name: np.asarray(outs[i]) for i, name in enumerate(ent["out_names"])}


def kernel(**inputs):
    inputs = {k: np.asarray(v) for k, v in inputs.items()}
    res = _run_fast(inputs)
    return res["out"].reshape(H)
